# revision 11
# baseline (speedup 1.0000x reference)
"""Trainium2 Bass kernel for a dense transformer block (attention + MLP),
data-parallel over the batch dimension across 8 NeuronCores.

Reference semantics (per batch element, computed fully on one core):
    x  = rms_norm(latents) * ln1_scale
    q,k,v = x @ wq, x @ wk, x @ wv          (heads H=16, head_dim D=64)
    q  = rms_norm_d(q) * q_ln_scale / sqrt(D);  k = rms_norm_d(k) * k_ln_scale
    o  = softmax(q k^T) v ;  o = o @ wo ;  x2 = o + latents
    y  = rms_norm(x2) * ln2_scale
    out = gelu(y @ wi) @ wo_mlp + x2

Key structural ideas vs a phase-serial implementation:
  * ln1_scale is folded into wq/wk/wv at bf16 conversion time (per-partition
    scale on the Act engine's Copy).  The per-token rms of x cancels inside
    the q/k rms-norms (up to eps), and is applied to v on its PSUM
    evacuation, so x itself is never normalized or re-transposed.
  * QKV projection of head-pair hp is interleaved with attention of
    head-pair hp-1 so the Act-engine exp stream hides under projection
    matmuls and the PE never idles (keeps the HAM clock gate open).
  * Attention works on 512-wide q chunks; PSUM = proj(2) + ssq(1) +
    logits(3) + AV accumulation(2) = 8 banks exactly.
  * Sum-of-squares over head_dim via one block-ones matmul per chunk.
  * MLP runs fc1/fc2 interleaved by groups of 8 F-tiles with an SBUF f32
    accumulator; wi/wo_mlp stream through a small fp32 staging pool.
"""

import os

import numpy as np

import concourse.bass as bass
import concourse.mybir as mybir
import concourse.tile as tile
from concourse import bacc
from concourse.bass_utils import run_bass_kernel_spmd
from concourse.masks import make_identity

F32 = mybir.dt.float32
BF16 = mybir.dt.bfloat16
AF = mybir.ActivationFunctionType
ALU = mybir.AluOpType

B, S, E, H, D, F = 8, 1024, 1024, 16, 64, 4096
HD = H * D            # 1024
ST = S // 128         # 8 token tiles
ET = E // 128         # 8 embedding tiles
FT = F // 128         # 32 mlp tiles
NCH = 512             # matmul moving-dim chunk (one psum bank of f32)
EPS = 1e-6


def build():
    nc = bacc.Bacc()

    lat_ext = nc.declare_dram_parameter("latents", [S, E], F32, isOutput=False)
    ln1_ext = nc.declare_dram_parameter("ln1_scale", [E], F32, isOutput=False)
    wq_ext = nc.declare_dram_parameter("wq", [E, HD], F32, isOutput=False)
    wk_ext = nc.declare_dram_parameter("wk", [E, HD], F32, isOutput=False)
    wv_ext = nc.declare_dram_parameter("wv", [E, HD], F32, isOutput=False)
    qls_ext = nc.declare_dram_parameter("q_ln_scale", [D], F32, isOutput=False)
    kls_ext = nc.declare_dram_parameter("k_ln_scale", [D], F32, isOutput=False)
    wo_ext = nc.declare_dram_parameter("wo", [HD, E], F32, isOutput=False)
    ln2_ext = nc.declare_dram_parameter("ln2_scale", [E], F32, isOutput=False)
    wi_ext = nc.declare_dram_parameter("wi", [E, F], F32, isOutput=False)
    wm_ext = nc.declare_dram_parameter("wo_mlp", [F, E], F32, isOutput=False)
    out_ext = nc.declare_dram_parameter("out", [S, E], F32, isOutput=True)

    with tile.TileContext(nc) as tc:
        # ---------------- constants (left, live to the end) --------------
        cst = tc.alloc_tile_pool(name="cst", bufs=1, side="left")
        ident = cst.tile([128, 128], BF16)
        make_identity(nc, ident[:])
        blkones = cst.tile([128, 65], BF16)
        nc.vector.memset(blkones[:], 0.0)
        nc.vector.memset(blkones[0:64, 0:1], 1.0)
        nc.vector.memset(blkones[64:128, 64:65], 1.0)
        ln1s = cst.tile([128, ET], F32)
        nc.sync.dma_start(ln1s[:], ln1_ext[:].rearrange("(t p) -> p t", p=128))
        ln2s = cst.tile([128, ET], F32)
        nc.sync.dma_start(ln2s[:], ln2_ext[:].rearrange("(t p) -> p t", p=128))
        qls2 = cst.tile([128, 1], F32)
        nc.sync.dma_start(qls2[0:64, :], qls_ext[:].rearrange("(d o) -> d o", o=1))
        nc.sync.dma_start(qls2[64:128, :], qls_ext[:].rearrange("(d o) -> d o", o=1))
        kls2 = cst.tile([128, 1], F32)
        nc.sync.dma_start(kls2[0:64, :], kls_ext[:].rearrange("(d o) -> d o", o=1))
        nc.sync.dma_start(kls2[64:128, :], kls_ext[:].rearrange("(d o) -> d o", o=1))
        c_eps = cst.tile([128, 1], F32)
        nc.vector.memset(c_eps[:], EPS)
        c_eps64 = cst.tile([128, 1], F32)
        nc.vector.memset(c_eps64[:], D * EPS)
        rinv = cst.tile([128, ST], F32)       # 1/rms(latents) per token
        rms_raw = cst.tile([128, 2 * ST], F32)
        r2v = cst.tile([128, ST], F32)        # 1/rms(x2) per token
        rms2_raw = cst.tile([128, 2 * ST], F32)

        # fp32 weight staging, shared by every weight load (left)
        stg = tc.alloc_tile_pool(name="stg", bufs=3, side="left")
        # o^T [HD, S] (left; released after o-proj)
        oT_p = tc.alloc_tile_pool(name="oT_p", bufs=1, side="left")
        oT = oT_p.tile([128, ET, S], BF16)
        # q/k/v weights bf16 (left; released before o-proj weight load)
        wqkv_p = tc.alloc_tile_pool(name="wqkv_p", bufs=1, side="left")
        wvb = [wqkv_p.tile([128, HD], BF16, name=f"wvb{i}") for i in range(ET)]
        wkb = [wqkv_p.tile([128, HD], BF16, name=f"wkb{i}") for i in range(ET)]
        wqb = [wqkv_p.tile([128, HD], BF16, name=f"wqb{i}") for i in range(ET)]

        # ---------------- right-side long-lived pools --------------------
        scr_p = tc.alloc_tile_pool(name="scr_p", bufs=4, side="right")
        latT_p = tc.alloc_tile_pool(name="latT_p", bufs=1, side="right")
        latT = latT_p.tile([128, ET, S], BF16)

        # ============ Phase 1: latents load, rinv, transpose =============
        latb_p = tc.alloc_tile_pool(name="latb_p", bufs=1, side="right")
        latb = latb_p.tile([128, ST, E], BF16)
        latld = tc.alloc_tile_pool(name="latld", bufs=2, side="right")
        p1ps = tc.alloc_tile_pool(name="p1ps", bufs=2, space="PSUM")

        for t in range(ST):
            lt = latld.tile([128, E], F32, tag="lat", name=f"lat{t}")
            nc.sync.dma_start(lt[:], lat_ext[t * 128:(t + 1) * 128, :])
            nc.vector.tensor_copy(latb[:, t, :], lt[:])
            sq = scr_p.tile([128, E], BF16, tag="sq1", name=f"sq1_{t}")
            nc.scalar.activation(sq[:], lt[:], AF.Square,
                                 accum_out=rms_raw[:, t:t + 1])
        # weight DMAs + conversions (Act Copy folds ln1_scale into wq/wk/wv)
        for wext, wsb in ((wv_ext, wvb), (wk_ext, wkb), (wq_ext, wqb)):
            for kk in range(ET):
                wt = stg.tile([128, HD], F32, tag="wstg", name=f"stg_{wext.name}{kk}")
                nc.sync.dma_start(wt[:], wext[kk * 128:(kk + 1) * 128, :])
                nc.scalar.activation(wsb[kk][:], wt[:], AF.Copy,
                                     scale=ln1s[:, kk:kk + 1])
        # rinv = 1/sqrt(mean(lat^2) + eps)
        nc.scalar.activation(rms_raw[:, ST:2 * ST], rms_raw[:, 0:ST],
                             AF.Sqrt, bias=c_eps[:], scale=1.0 / E)
        nc.vector.reciprocal(rinv[:], rms_raw[:, ST:2 * ST])
        # transpose latents -> latT
        for e in range(ET):
            tp = p1ps.tile([128, S], BF16, tag="tp1", name=f"tp1_{e}")
            for t in range(ST):
                nc.tensor.transpose(tp[:, t * 128:(t + 1) * 128],
                                    latb[:, t, e * 128:(e + 1) * 128], ident[:])
            nc.vector.tensor_copy(latT[:, e, :], tp[:])

        p1ps.release()
        latld.release()
        latb_p.release()

        # ============ Pass 1: v projection ([S, HD] + ones col) ==========
        qkv_p = tc.alloc_tile_pool(name="qkv_p", bufs=1, side="right")
        qT = qkv_p.tile([128, ET, S], BF16)
        kT = qkv_p.tile([128, ET, S], BF16)
        v_sb = qkv_p.tile([128, ST, H * (D + 1)], BF16)
        v3 = v_sb[:].rearrange("p t (h c) -> p t h c", c=D + 1)

        p1vps = tc.alloc_tile_pool(name="p1vps", bufs=2, space="PSUM")
        for skt in range(ST):
            ps = p1vps.tile([128, HD], F32, tag="vps", name=f"vps{skt}")
            for kk in range(ET):
                for c in range(2):
                    ch = slice(c * NCH, (c + 1) * NCH)
                    nc.tensor.matmul(
                        ps[:, ch],
                        latT[:, kk, skt * 128:(skt + 1) * 128],
                        wvb[kk][:, ch],
                        start=(kk == 0), stop=(kk == ET - 1))
            nc.vector.tensor_scalar_mul(
                v3[:, skt, :, 0:D],
                ps[:].rearrange("p (h c) -> p h c", c=D),
                rinv[:, skt:skt + 1])
            nc.vector.memset(v3[:, skt, :, D:D + 1], 1.0)
        p1vps.release()

        # ============ Pass 2: q/k proj interleaved with attention ========
        rq_p = tc.alloc_tile_pool(name="rq_p", bufs=2, side="right")
        bc_p = tc.alloc_tile_pool(name="bc_p", bufs=6, side="right")
        rs_p = tc.alloc_tile_pool(name="rs_p", bufs=2, side="right")
        exp_p = tc.alloc_tile_pool(name="exp_p", bufs=2, side="right")

        kqps = tc.alloc_tile_pool(name="kqps", bufs=2, space="PSUM")
        ssqps = tc.alloc_tile_pool(name="ssqps", bufs=1, space="PSUM")
        lgps = tc.alloc_tile_pool(name="lgps", bufs=3, space="PSUM")
        oaps = tc.alloc_tile_pool(name="oaps", bufs=2, space="PSUM")

        def proj_block(hp):
            """q/k projection for head pair hp + rms_d norms -> qT/kT."""
            for pname, wsb, outT, scl, sc in (
                ("k", wkb, kT, kls2, 1.0 / D), ("q", wqb, qT, qls2, 1.0),
            ):
                bi = c_eps if pname == "k" else c_eps64
                for c in range(2):
                    ch = slice(c * NCH, (c + 1) * NCH)
                    ps = kqps.tile([128, NCH], F32, tag="kq",
                                   name=f"kq{pname}{hp}_{c}")
                    for kk in range(ET):
                        nc.tensor.matmul(
                            ps[:],
                            wsb[kk][:, hp * 128:(hp + 1) * 128],
                            latT[:, kk, ch],
                            start=(kk == 0), stop=(kk == ET - 1))
                    scr = scr_p.tile([128, NCH], BF16, tag="sq2",
                                     name=f"sq2{pname}{hp}_{c}")
                    nc.scalar.activation(scr[:], ps[:], AF.Square)
                    ssq = ssqps.tile([65, NCH], F32, tag="ssq",
                                     name=f"ssq{pname}{hp}_{c}")
                    nc.tensor.matmul(ssq[:], blkones[:], scr[:],
                                     start=True, stop=True)
                    ssqb = rq_p.tile([1, NCH], F32, tag="ssqb",
                                     name=f"ssqb{pname}{hp}_{c}")
                    nc.vector.tensor_copy(ssqb[:], ssq[64:65, :])
                    rqa = rq_p.tile([1, NCH], F32, tag="rqa",
                                    name=f"rqa{pname}{hp}_{c}")
                    nc.scalar.activation(rqa[:], ssq[0:1, :],
                                         AF.Abs_reciprocal_sqrt,
                                         bias=bi[0:1, :], scale=sc)
                    rqb = rq_p.tile([1, NCH], F32, tag="rqb",
                                    name=f"rqb{pname}{hp}_{c}")
                    nc.scalar.activation(rqb[:], ssqb[:],
                                         AF.Abs_reciprocal_sqrt,
                                         bias=bi[0:1, :], scale=sc)
                    bcA = bc_p.tile([64, NCH], F32, tag="bc",
                                    name=f"bcA{pname}{hp}_{c}")
                    nc.gpsimd.partition_broadcast(bcA[:], rqa[:])
                    bcB = bc_p.tile([64, NCH], F32, tag="bc",
                                    name=f"bcB{pname}{hp}_{c}")
                    nc.gpsimd.partition_broadcast(bcB[:], rqb[:])
                    nc.vector.scalar_tensor_tensor(
                        out=outT[0:64, hp, ch], in0=ps[0:64, :],
                        scalar=scl[0:64, :], in1=bcA[:],
                        op0=ALU.mult, op1=ALU.mult)
                    nc.vector.scalar_tensor_tensor(
                        out=outT[64:128, hp, ch], in0=ps[64:128, :],
                        scalar=scl[0:64, :], in1=bcB[:],
                        op0=ALU.mult, op1=ALU.mult)

        def attn_block(hp):
            """logits+softmax+AV for heads 2hp, 2hp+1 (q in 512 chunks)."""
            for half in range(2):
                h = 2 * hp + half
                hs = slice(half * 64, (half + 1) * 64)
                for qc in range(2):
                    qch = slice(qc * NCH, (qc + 1) * NCH)
                    eb = exp_p.tile([128, ST, NCH], BF16, tag="exp",
                                    name=f"exp{h}_{qc}")
                    oa = oaps.tile([D + 1, NCH], F32, tag="oa",
                                   name=f"oa{h}_{qc}")

                    def av(skt):
                        nc.tensor.matmul(
                            oa[:], v3[:, skt, h, :], eb[:, skt, :],
                            start=(skt == 0), stop=(skt == ST - 1))

                    for skt in range(ST):
                        lg = lgps.tile([128, NCH], F32, tag="lg",
                                       name=f"lg{h}_{qc}_{skt}")
                        nc.tensor.matmul(
                            lg[:],
                            kT[hs, hp, skt * 128:(skt + 1) * 128],
                            qT[hs, hp, qch],
                            start=True, stop=True)
                        nc.scalar.activation(eb[:, skt, :], lg[:], AF.Exp)
                        if skt >= 3:
                            av(skt - 3)
                    for skt in range(ST - 3, ST):
                        av(skt)
                    rs = rs_p.tile([1, NCH], F32, tag="rs", name=f"rs{h}_{qc}")
                    nc.vector.reciprocal(rs[:], oa[D:D + 1, :])
                    bco = bc_p.tile([64, NCH], F32, tag="bc", name=f"bco{h}_{qc}")
                    nc.gpsimd.partition_broadcast(bco[:], rs[:])
                    nc.vector.tensor_tensor(
                        oT[hs, hp, qch], oa[0:D, :], bco[:], ALU.mult)

        proj_block(0)
        for hp in range(1, ET):
            attn_block(hp - 1)
            proj_block(hp)
        attn_block(ET - 1)

        oaps.release()
        lgps.release()
        ssqps.release()
        kqps.release()
        exp_p.release()
        rs_p.release()
        bc_p.release()
        rq_p.release()
        qkv_p.release()
        latT_p.release()
        wqkv_p.release()

        # ============ Phase 3: o-proj + residual + ln2 + transpose =======
        # wo loads into the space wqkv released (left)
        wo_p = tc.alloc_tile_pool(name="wo_p", bufs=1, side="left")
        wob = [wo_p.tile([128, E], BF16, name=f"wob{i}") for i in range(ET)]
        for kk in range(ET):
            wt = stg.tile([128, E], F32, tag="wstg", name=f"stg_wo{kk}")
            nc.sync.dma_start(wt[:], wo_ext[kk * 128:(kk + 1) * 128, :])
            nc.vector.tensor_copy(wob[kk][:], wt[:])

        x2_p = tc.alloc_tile_pool(name="x2_p", bufs=1, side="right")
        x2 = x2_p.tile([128, ST, E], BF16)
        yT_p = tc.alloc_tile_pool(name="yT_p", bufs=1, side="right")
        yT = yT_p.tile([128, ET, S], BF16)
        # wi group weights (bf16, ln2-folded), 2 groups in flight
        wi_p = tc.alloc_tile_pool(name="wi_p", bufs=2, side="right")
        # raw latents reload for the residual
        latr_p = tc.alloc_tile_pool(name="latr_p", bufs=1, side="right")
        latr = latr_p.tile([128, ST, E], F32)
        y_p = tc.alloc_tile_pool(name="y_p", bufs=1, side="right")
        y = y_p.tile([128, ST, E], BF16)

        for t in range(ST):
            nc.sync.dma_start(latr[:, t, :], lat_ext[t * 128:(t + 1) * 128, :])

        # prefetch + convert wi groups 0..1 (execute during o-proj)
        wig = []
        for g in range(4):
            wig.append(wi_p.tile([128, ET, 1024], BF16, tag="wig",
                                 name=f"wig{g}"))
        def wi_conv(g):
            for kk in range(ET):
                wt = stg.tile([128, 1024], F32, tag="wstg",
                              name=f"stg_wi{g}_{kk}")
                nc.sync.dma_start(
                    wt[:], wi_ext[kk * 128:(kk + 1) * 128,
                                  g * 1024:(g + 1) * 1024])
                eng = nc.scalar if kk % 2 == 0 else nc.vector
                if eng is nc.scalar:
                    nc.scalar.activation(wig[g][:, kk, :], wt[:], AF.Copy,
                                         scale=ln2s[:, kk:kk + 1])
                else:
                    nc.vector.tensor_scalar_mul(wig[g][:, kk, :], wt[:],
                                                ln2s[:, kk:kk + 1])
        wi_conv(0)
        wi_conv(1)

        p3ps = tc.alloc_tile_pool(name="p3ps", bufs=2, space="PSUM")
        for m in range(ST):
            ps = p3ps.tile([128, E], F32, tag="op", name=f"op{m}")
            for kk in range(ET):
                for c in range(2):
                    ch = slice(c * NCH, (c + 1) * NCH)
                    nc.tensor.matmul(
                        ps[:, ch],
                        oT[:, kk, m * 128:(m + 1) * 128],
                        wob[kk][:, ch],
                        start=(kk == 0), stop=(kk == ET - 1))
            nc.vector.tensor_tensor(x2[:, m, :], ps[:], latr[:, m, :], ALU.add)
            sq = scr_p.tile([128, E], BF16, tag="sq1", name=f"sq3_{m}")
            nc.scalar.activation(sq[:], x2[:, m, :], AF.Square,
                                 accum_out=rms2_raw[:, m:m + 1])
        nc.scalar.activation(rms2_raw[:, ST:2 * ST], rms2_raw[:, 0:ST],
                             AF.Sqrt, bias=c_eps[:], scale=1.0 / E)
        nc.vector.reciprocal(r2v[:], rms2_raw[:, ST:2 * ST])
        for m in range(ST):
            nc.vector.tensor_scalar_mul(y[:, m, :], x2[:, m, :],
                                        r2v[:, m:m + 1])
        p3tps = tc.alloc_tile_pool(name="p3tps", bufs=2, space="PSUM")
        for e in range(ET):
            tp = p3tps.tile([128, S], BF16, tag="tp3", name=f"tp3_{e}")
            for t in range(ST):
                nc.tensor.transpose(tp[:, t * 128:(t + 1) * 128],
                                    y[:, t, e * 128:(e + 1) * 128], ident[:])
            nc.vector.tensor_copy(yT[:, e, :], tp[:])
        p3tps.release()
        p3ps.release()
        y_p.release()
        latr_p.release()

        # ============ Phase 4: MLP (fc1/fc2 interleaved by F-group) ======
        wo_p.release()
        oT_p.release()
        wm_p = tc.alloc_tile_pool(name="wm_p", bufs=2, side="left")
        h1_p = tc.alloc_tile_pool(name="h1_p", bufs=2, side="right")
        acc_p = tc.alloc_tile_pool(name="acc_p", bufs=1, side="right")
        acc = acc_p.tile([128, ST, E], F32)
        out_p = tc.alloc_tile_pool(name="out_p", bufs=2, side="right")

        f1ps = tc.alloc_tile_pool(name="f1ps", bufs=2, space="PSUM")
        f2ps = tc.alloc_tile_pool(name="f2ps", bufs=2, space="PSUM")

        h1g = [h1_p.tile([128, ET, S], BF16, tag="h1", name=f"h1g{g}")
               for g in range(4)]
        wmg = [wm_p.tile([128, ET, E], BF16, tag="wmg", name=f"wmg{g}")
               for g in range(4)]

        def wm_conv(g):
            for kk in range(ET):
                kf = g * ET + kk
                wt = stg.tile([128, E], F32, tag="wstg", name=f"stg_wm{kf}")
                nc.sync.dma_start(wt[:], wm_ext[kf * 128:(kf + 1) * 128, :])
                nc.vector.tensor_copy(wmg[g][:, kk, :], wt[:])

        def fc1(g):
            for mf in range(ET):
                ps = f1ps.tile([128, S], F32, tag="f1", name=f"f1_{g}_{mf}")
                for kk in range(ET):
                    for c in range(2):
                        ch = slice(c * NCH, (c + 1) * NCH)
                        nc.tensor.matmul(
                            ps[:, ch],
                            wig[g][:, kk, mf * 128:(mf + 1) * 128],
                            yT[:, kk, ch],
                            start=(kk == 0), stop=(kk == ET - 1))
                nc.scalar.activation(h1g[g][:, mf, :], ps[:],
                                     AF.Gelu_apprx_tanh)

        def fc2(g):
            for ms in range(ST):
                ps = f2ps.tile([128, E], F32, tag="f2", name=f"f2_{g}_{ms}")
                for kk in range(ET):
                    for c in range(2):
                        ch = slice(c * NCH, (c + 1) * NCH)
                        nc.tensor.matmul(
                            ps[:, ch],
                            h1g[g][:, kk, ms * 128:(ms + 1) * 128],
                            wmg[g][:, kk, ch],
                            start=(kk == 0), stop=(kk == ET - 1))
                if g == 0:
                    nc.vector.tensor_tensor(acc[:, ms, :], ps[:], x2[:, ms, :],
                                            ALU.add)
                elif g < 3:
                    nc.vector.tensor_tensor(acc[:, ms, :], ps[:], acc[:, ms, :],
                                            ALU.add)
                else:
                    ot = out_p.tile([128, E], F32, tag="out", name=f"out{ms}")
                    nc.vector.tensor_tensor(ot[:], ps[:], acc[:, ms, :],
                                            ALU.add)
                    nc.sync.dma_start(out_ext[ms * 128:(ms + 1) * 128, :],
                                      ot[:])

        wm_conv(0)
        fc1(0)
        wi_conv(2)
        wm_conv(1)
        fc1(1)
        fc2(0)
        wi_conv(3)
        wm_conv(2)
        fc1(2)
        fc2(1)
        wm_conv(3)
        fc1(3)
        fc2(2)
        fc2(3)

        f2ps.release()
        f1ps.release()
        out_p.release()
        acc_p.release()
        h1_p.release()
        wm_p.release()
        wi_p.release()
        yT_p.release()
        x2_p.release()
        scr_p.release()
        stg.release()
        cst.release()

    nc.finalize()
    return nc


_NC_CACHE = None


def kernel(**inputs) -> np.ndarray:
    global _NC_CACHE
    if _NC_CACHE is None:
        _NC_CACHE = build()
    nc = _NC_CACHE

    f32 = lambda a: np.ascontiguousarray(np.asarray(a), dtype=np.float32)
    base = {
        "ln1_scale": f32(inputs["ln1_scale"]),
        "wq": f32(inputs["wq"]).reshape(E, HD),
        "wk": f32(inputs["wk"]).reshape(E, HD),
        "wv": f32(inputs["wv"]).reshape(E, HD),
        "q_ln_scale": f32(inputs["q_ln_scale"]),
        "k_ln_scale": f32(inputs["k_ln_scale"]),
        "wo": f32(inputs["wo"]).reshape(HD, E),
        "ln2_scale": f32(inputs["ln2_scale"]),
        "wi": f32(inputs["wi"]),
        "wo_mlp": f32(inputs["wo_mlp"]),
    }
    lat = f32(inputs["latents"])
    in_maps = [dict(base, latents=np.ascontiguousarray(lat[i])) for i in range(B)]
    res = run_bass_kernel_spmd(nc, in_maps, list(range(B)))
    return np.stack([res.results[i]["out"] for i in range(B)], axis=0)
